# revision 30
# baseline (speedup 1.0000x reference)
"""ConditionalRandomField loss kernel for Trainium2 (8 NeuronCores).

Math (per sequence b):
    loss[b] = log_score(gold path) - log_partition
The log-partition is computed in exp space from BOTH ENDS simultaneously:
    forward   alpha_t = g_t * (E^T alpha_{t-1}),  alpha_0 = exp(start)*g_0
    backward  beta_t  = E (g_{t+1} * beta_{t+1}), beta_{L-1} = exp(stop)
    Z = alpha_m . beta_m   at the meeting point m = L/2 - 1
with E = exp(transitions - CSH), g_t = exp(emissions_t), and periodic
per-batch rescaling whose log is accumulated separately.  Halving the
sequential depth doubles the number of independent recurrence streams
(2 directions x 2 batch-chains = 4), which is what hides the per-step
PE-drain -> multiply -> matmul latency.

Engine split: forward-stream f-updates run on the Vector (DVE) engine,
backward-stream updates on the GpSimd (Pool) engine, so neither engine
serializes both directions.

The gold-path numerator uses one-hot tag masks in b-major layout
(OH[p, b*1025 + t]) so every multiply-accumulate is a contiguous
scalar_tensor_tensor; emissions are uploaded a second time in
[p, jc, b, t] order for the same reason.  All numerator work runs after
the scan (tail) so the scan's first step starts as early as possible.

NOTE: mask is all-ones for this problem spec (fill: ones); the kernel
assumes it (the reference's masked branches are identities then).
"""

import numpy as np
from contextlib import ExitStack

import concourse.bass as bass
import concourse.bacc as bacc
import concourse.tile as tile
from concourse import mybir
from concourse.bass_utils import run_bass_kernel_spmd

F32 = mybir.dt.float32
BF16 = mybir.dt.bfloat16

NCORES = 8
B = 64
L = 1024
T = 256
BC = B // NCORES      # sequences per core
PJ = 128              # partition tile of the tag dim
JCN = T // PJ         # = 2 tag chunks
RS = 32               # rescale sampling period (steps)
DEFER = 2             # rescale applied this many steps after sampling
TCH = 128             # emission-load chunk (timesteps per DMA/exp chunk)
CSH = 6.5             # constant log-shift folded into E = exp(transitions - CSH)
M = L // 2 - 1        # meeting point: fwd owns t<=M, bwd owns t>M

LP = L + 1            # per-b one-hot row width (padded col at t=L)

AUX_TRT = T * T                  # transitions^T - CSH
AUX_START = 2 * T * T
AUX_STOP = 2 * T * T + T
AUX_TRRAW = 2 * T * T + 2 * T    # unshifted transitions (numerator)
AUX_N = 3 * T * T + 2 * T

NCH = 1               # batch chains per direction (merged: width-8 matmuls)
BCH = BC // NCH


def _sample_steps(nsteps):
    # loop iterations k in [1, nsteps] at which to sample the running scale
    return [k for k in range(1, nsteps + 1) if k % RS == 0 and k + DEFER <= nsteps]


class _Bacc(bacc.Bacc):
    # Keep data waits on the MATMULs so the (data-independent) LDWEIGHTS
    # prefetch during the preceding multiply phase instead of stalling.
    def move_matmul_waits_to_ldweights(self):
        super().move_matmul_waits_to_ldweights()


def build_program(length=L, pool_bwd=True):
    """Build the single-core SPMD bass program (each core runs the same
    program on its own batch shard)."""
    assert length % 2 == 0
    m = length // 2 - 1          # fwd steps: t=1..m ; bwd steps: 1+m more
    nsamp = len(_sample_steps(m))
    nspad = max(8, ((nsamp + 7) // 8) * 8)

    nc = _Bacc()
    em_t = nc.declare_dram_parameter("em", [BC * length * T, 1], F32, isOutput=False)
    em2_t = nc.declare_dram_parameter("em2", [BC * length * T, 1], F32, isOutput=False)
    aux_t = nc.declare_dram_parameter("aux", [AUX_N, 1], F32, isOutput=False)
    tags_t = nc.declare_dram_parameter("tags_bt", [BC * LP, 1], F32, isOutput=False)
    iota_t = nc.declare_dram_parameter("iota", [128, 1], F32, isOutput=False)
    loss_t = nc.declare_dram_parameter("loss", [BC, 1], F32, isOutput=True)

    def dram_ap(handle, offset, ap):
        full = handle[:]
        return bass.AP(tensor=full.tensor, offset=offset, ap=ap)

    with tile.TileContext(nc) as tc, ExitStack() as ctx:
        const = ctx.enter_context(tc.tile_pool(name="const", bufs=1))
        stage = ctx.enter_context(tc.tile_pool(name="stage", bufs=2))
        gpool = ctx.enter_context(tc.tile_pool(name="gpool", bufs=1))
        fpool = ctx.enter_context(tc.tile_pool(name="fpool", bufs=3))
        wpool = ctx.enter_context(tc.tile_pool(name="wpool", bufs=3))
        qpool = ctx.enter_context(tc.tile_pool(name="qpool", bufs=3))
        vpool = ctx.enter_context(tc.tile_pool(name="vpool", bufs=2))
        ppool = ctx.enter_context(tc.tile_pool(name="ppool", bufs=2, space="PSUM"))
        pbpool = ctx.enter_context(tc.tile_pool(name="pbpool", bufs=2, space="PSUM"))
        spool = ctx.enter_context(tc.tile_pool(name="spool", bufs=2, space="PSUM"))
        smallp = ctx.enter_context(tc.tile_pool(name="smallp", bufs=2, space="PSUM"))

        bwd = nc.gpsimd if pool_bwd else nc.vector

        # ---------------- head emission blocks first --------------------
        # The scan's first steps only need g at the two sequence ends; load
        # tiny 16-step head blocks before anything else so the recurrences
        # start ~2us in instead of waiting for full 1MB chunks.
        gbuf = gpool.tile([128, length, JCN, BC], BF16, name="gbuf")
        row = length * JCN * BC
        HL = 16
        for hoff in (0, length - HL):
            hraw = stage.tile([128, HL, JCN, BC], F32, name="hraw", tag="raw")
            nc.sync.dma_start(
                out=hraw,
                in_=dram_ap(em_t, hoff * JCN * BC, [[row, 128], [1, HL * JCN * BC]]),
            )
            nc.scalar.activation(
                out=gbuf[:, hoff : hoff + HL, :, :],
                in_=hraw,
                func=mybir.ActivationFunctionType.Exp,
            )

        # ---------------- constants / setup ----------------
        # E = exp(transitions - CSH) / exp(transitions^T - CSH), bf16.
        e_fwd, e_bwd, tr_tiles = [], [], []
        for ic in range(JCN):
            eraw = stage.tile([128, T], F32, name=f"eraw{ic}", tag="eraw")
            nc.sync.dma_start(
                out=eraw, in_=dram_ap(aux_t, ic * 128 * T, [[T, 128], [1, T]])
            )
            ebf = const.tile([128, T], BF16, name=f"ebf{ic}")
            nc.scalar.activation(
                out=ebf, in_=eraw, func=mybir.ActivationFunctionType.Exp
            )
            e_fwd.append(ebf)
            erawt = stage.tile([128, T], F32, name=f"erawt{ic}", tag="eraw")
            nc.sync.dma_start(
                out=erawt,
                in_=dram_ap(aux_t, AUX_TRT + ic * 128 * T, [[T, 128], [1, T]]),
            )
            ebft = const.tile([128, T], BF16, name=f"ebft{ic}")
            nc.scalar.activation(
                out=ebft, in_=erawt, func=mybir.ActivationFunctionType.Exp
            )
            e_bwd.append(ebft)

        # exp(start), exp(stop) as per-partition f32 scales [128, JCN]
        ssraw = stage.tile([128, 2 * JCN], F32, name="ssraw", tag="ss")
        nc.sync.dma_start(
            out=ssraw[:, 0:JCN], in_=dram_ap(aux_t, AUX_START, [[1, 128], [128, JCN]])
        )
        nc.sync.dma_start(
            out=ssraw[:, JCN : 2 * JCN],
            in_=dram_ap(aux_t, AUX_STOP, [[1, 128], [128, JCN]]),
        )
        sstart = const.tile([128, JCN], F32, name="sstart")
        nc.scalar.activation(
            out=sstart, in_=ssraw[:, 0:JCN], func=mybir.ActivationFunctionType.Exp
        )
        sstop = const.tile([128, JCN], F32, name="sstop")
        nc.scalar.activation(
            out=sstop, in_=ssraw[:, JCN : 2 * JCN], func=mybir.ActivationFunctionType.Exp
        )
        # raw start/stop bf16 for the numerator matmuls
        ssbf = const.tile([128, 2 * JCN], BF16, name="ssbf")
        nc.vector.tensor_copy(out=ssbf, in_=ssraw)

        ones_w = const.tile([128, 128], BF16, name="ones_w")
        nc.vector.memset(ones_w, 1.0)
        ones_col = const.tile([128, 1], BF16, name="ones_col")
        nc.vector.memset(ones_col, 1.0)
        ones_col_f = const.tile([128, 1], F32, name="ones_col_f")
        nc.vector.memset(ones_col_f, 1.0)

        iota_sb = const.tile([128, 1], F32, name="iota_sb")
        nc.sync.dma_start(out=iota_sb, in_=iota_t[:])
        # tags in b-major order, padded with -1 at t=L (never matches a tag)
        tags_bt = const.tile([128, BC * LP], F32, name="tags_bt")
        nc.sync.dma_start(
            out=tags_bt, in_=dram_ap(tags_t, 0, [[0, 128], [1, BC * LP]])
        )

        # ---------------- emissions -> g = exp(emissions), bf16 ------------
        # em is host-pretransposed to [p, t, jc, b]; chunks loaded ends-first
        # so both scan directions can start early.  em2 ([p, jc, b, t]) is
        # only consumed by the tail numerator.
        em2 = gpool.tile([128, JCN, BC, length], F32, name="em2")
        nct = length // TCH
        order = []
        lo, hi = 0, nct - 1
        while lo <= hi:
            order.append(lo)
            if hi != lo:
                order.append(hi)
            lo += 1
            hi -= 1
        for tci in order:
            # skip the 16-step head blocks already loaded above
            lo_t = tci * TCH + (HL if tci == 0 else 0)
            hi_t = (tci + 1) * TCH - (HL if tci == nct - 1 else 0)
            nt = hi_t - lo_t
            raw = stage.tile([128, nt, JCN, BC], F32, name="raw", tag="raw")
            nc.sync.dma_start(
                out=raw,
                in_=dram_ap(em_t, lo_t * JCN * BC, [[row, 128], [1, nt * JCN * BC]]),
            )
            nc.scalar.activation(
                out=gbuf[:, lo_t:hi_t, :, :],
                in_=raw,
                func=mybir.ActivationFunctionType.Exp,
            )
        # em2 is contiguous per partition-row; 4 fat transfers
        for c in range(4):
            nc.sync.dma_start(
                out=bass.AP(
                    tensor=em2.tensor,
                    offset=em2.offset + c * 4 * length,
                    ap=[em2.ap[0], [1, 4 * length]],
                ),
                in_=dram_ap(em2_t, c * 4 * length, [[row, 128], [1, 4 * length]]),
            )

        # ---------------- the scan: 4 independent streams ------------------
        bsl = [slice(ch * BCH, (ch + 1) * BCH) for ch in range(NCH)]

        # forward init: alpha_0 = exp(start) * g_0
        fs = []
        for ch in range(NCH):
            f = fpool.tile([128, JCN, BCH], BF16, name=f"f{ch}", tag=f"f{ch}")
            for jc in range(JCN):
                nc.vector.tensor_scalar_mul(
                    out=f[:, jc, :],
                    in0=gbuf[:, 0, jc, bsl[ch]],
                    scalar1=sstart[:, jc : jc + 1],
                )
            fs.append(f)
        # backward init: w_0 = g_{L-1} * exp(stop)  (the TT of loop k=0)
        ws = []
        for ch in range(NCH):
            w = wpool.tile([128, JCN, BCH], BF16, name=f"w{ch}", tag=f"w{ch}")
            for jc in range(JCN):
                # on DVE: the Pool/Q7 version of this init measured 4.1us
                # and gated the entire scan start
                nc.vector.tensor_scalar_mul(
                    out=w[:, jc, :],
                    in0=gbuf[:, length - 1, jc, bsl[ch]],
                    scalar1=sstop[:, jc : jc + 1],
                )
            ws.append(w)

        logsf = []
        logsb = []
        for ch in range(NCH):
            lf = const.tile([BCH, nspad], F32, name=f"logsf{ch}")
            nc.vector.memset(lf, 1.0)  # log(1)=0 padding
            logsf.append(lf)
            lb = const.tile([BCH, nspad], F32, name=f"logsb{ch}")
            nc.vector.memset(lb, 1.0)
            logsb.append(lb)

        def fwd_step(ch, t, f, p):
            """alpha: p = E^T f ; f' = p * g_t   (TT on vector)."""
            for jc in range(JCN):
                nc.tensor.matmul(
                    out=p[:, ch, jc, :],
                    lhsT=e_fwd[0][:, jc * 128 : (jc + 1) * 128],
                    rhs=f[:, 0, :],
                    start=True,
                    stop=False,
                )
                nc.tensor.matmul(
                    out=p[:, ch, jc, :],
                    lhsT=e_fwd[1][:, jc * 128 : (jc + 1) * 128],
                    rhs=f[:, 1, :],
                    start=False,
                    stop=True,
                )
            fn = fpool.tile([128, JCN, BCH], BF16, name=f"f{ch}", tag=f"f{ch}")
            nc.vector.tensor_tensor(
                out=fn[:],
                in0=p[:, ch],
                in1=gbuf[:, t, :, bsl[ch]],
                op=mybir.AluOpType.mult,
            )
            return fn

        def bwd_step(ch, t, w, q):
            """beta: q = E w  (w = g*beta); w' = q * g_t  (TT on pool)."""
            for ic in range(JCN):
                nc.tensor.matmul(
                    out=q[:, ch, ic, :],
                    lhsT=e_bwd[0][:, ic * 128 : (ic + 1) * 128],
                    rhs=w[:, 0, :],
                    start=True,
                    stop=False,
                )
                nc.tensor.matmul(
                    out=q[:, ch, ic, :],
                    lhsT=e_bwd[1][:, ic * 128 : (ic + 1) * 128],
                    rhs=w[:, 1, :],
                    start=False,
                    stop=True,
                )
            # With merged chains there are only 2 TTs per dual-step, so DVE
            # reads PSUM directly (no ACT hop: every cross-engine handoff
            # costs 300-600ns in standalone semaphore waits).
            wn = wpool.tile([128, JCN, BCH], BF16, name=f"w{ch}", tag=f"w{ch}")
            nc.vector.tensor_tensor(
                out=wn[:],
                in0=q[:, ch],
                in1=gbuf[:, t, :, bsl[ch]],
                op=mybir.AluOpType.mult,
            )
            return wn

        def sample(ch, x, eng, logbuf, col, gslice, dtag):
            """Sample sum(x) per sequence, store log, rescale future g."""
            s_bc = spool.tile([128, BCH], F32, name="s_bc", tag="s")
            nc.tensor.matmul(
                out=s_bc, lhsT=ones_w, rhs=x[:, 0, :], start=True, stop=False
            )
            nc.tensor.matmul(
                out=s_bc, lhsT=ones_w, rhs=x[:, 1, :], start=False, stop=True
            )
            s4 = smallp.tile([BCH, 1], F32, name="s4", tag="small")
            nc.tensor.matmul(
                out=s4, lhsT=x[:, 0, :], rhs=ones_col, start=True, stop=False
            )
            nc.tensor.matmul(
                out=s4, lhsT=x[:, 1, :], rhs=ones_col, start=False, stop=True
            )
            v = vpool.tile([128, BCH], F32, name="v", tag=f"v{dtag}{ch}")
            nc.vector.reciprocal(out=v, in_=s_bc)
            # s4 is PSUM: Pool can't read it; ACT does the copy either way
            nc.scalar.activation(
                out=logbuf[:, col : col + 1],
                in_=s4,
                func=mybir.ActivationFunctionType.Copy,
            )
            for jc in range(JCN):
                eng.tensor_mul(
                    out=gbuf[:, gslice, jc, bsl[ch]],
                    in0=gbuf[:, gslice, jc, bsl[ch]],
                    in1=v,
                )

        msamp = 0
        for k in range(1, m + 1):
            do_samp = k % RS == 0 and k + DEFER <= m
            p = ppool.tile([128, NCH, JCN, BCH], F32, name="pf", tag="pf")
            q = pbpool.tile([128, NCH, JCN, BCH], F32, name="pb", tag="pb")
            for ch in range(NCH):
                fs[ch] = fwd_step(ch, k, fs[ch], p)
                ws[ch] = bwd_step(ch, length - 1 - k, ws[ch], q)
            if do_samp:
                for ch in range(NCH):
                    sample(ch, fs[ch], nc.vector, logsf[ch], msamp, k + DEFER, "f")
                    sample(
                        ch,
                        ws[ch],
                        nc.vector,
                        logsb[ch],
                        msamp,
                        length - 1 - k - DEFER,
                        "b",
                    )
                msamp += 1
        assert msamp == nsamp

        # Final backward matmul: beta_m = E w  (no trailing multiply).
        # Total E applications: m (fwd) + m+1 (bwd) = 2m+1 = length-1.
        qfin = pbpool.tile([128, NCH, JCN, BCH], F32, name="pb", tag="pb")
        for ch in range(NCH):
            for ic in range(JCN):
                nc.tensor.matmul(
                    out=qfin[:, ch, ic, :],
                    lhsT=e_bwd[0][:, ic * 128 : (ic + 1) * 128],
                    rhs=ws[ch][:, 0, :],
                    start=True,
                    stop=False,
                )
                nc.tensor.matmul(
                    out=qfin[:, ch, ic, :],
                    lhsT=e_bwd[1][:, ic * 128 : (ic + 1) * 128],
                    rhs=ws[ch][:, 1, :],
                    start=False,
                    stop=True,
                )

        # join: Z = sum_j alpha_m[j] * beta_m[j]
        prods = []
        for ch in range(NCH):
            prod = const.tile([128, JCN, BCH], BF16, name=f"prod{ch}")
            nc.vector.tensor_tensor(
                out=prod[:], in0=qfin[:, ch], in1=fs[ch][:], op=mybir.AluOpType.mult
            )
            prods.append(prod)

        # ---------------- numerator (tail): one-hot masks -------------------
        # OH_jc[p, b*LP + t] = 1.0 iff tags[b, t] == jc*128 + p (bf16);
        # the padded tag value -1 at t=L gives a zero column automatically.
        # built on GpSimd: it idles during the scan, and the scheduler
        # hoists these 4.5us builds ahead of the scan TTs if put on DVE
        oh_tiles = []
        for jc in range(JCN):
            oh = const.tile([128, BC * LP], BF16, name=f"oh{jc}")
            bwd.tensor_scalar(
                out=oh,
                in0=tags_bt,
                scalar1=float(jc * 128),
                scalar2=iota_sb[:],
                op0=mybir.AluOpType.subtract,
                op1=mybir.AluOpType.is_equal,
            )
            oh_tiles.append(oh)

        # emission part: acc2e[:, (jc,b)] = sum_t em2[p, jc, b, t]*OH[p, b*LP+t]
        ne_calls = JCN * BC
        acc2e = const.tile([128, ne_calls], F32, name="acc2e")
        scr_v = const.tile([128, length], BF16, name="scr_v")
        scr_p = const.tile([128, length], BF16, name="scr_p")
        for jc in range(JCN):
            for b in range(BC):
                em_ap = bass.AP(
                    tensor=em2.tensor,
                    offset=em2.offset + (jc * BC + b) * length,
                    ap=[em2.ap[0], [1, length]],
                )
                oh_ap = bass.AP(
                    tensor=oh_tiles[jc].tensor,
                    offset=oh_tiles[jc].offset + b * LP,
                    ap=[oh_tiles[jc].ap[0], [1, length]],
                )
                acc_ap = acc2e[:, jc * BC + b : jc * BC + b + 1]
                if b % 2 == 0:
                    nc.vector.scalar_tensor_tensor(
                        out=scr_v[:, 0:length],
                        in0=em_ap,
                        scalar=1.0,
                        in1=oh_ap,
                        op0=mybir.AluOpType.mult,
                        op1=mybir.AluOpType.mult,
                        accum_out=acc_ap,
                    )
                else:
                    # Pool has no accumulator: multiply there, reduce on DVE
                    bwd.tensor_tensor(
                        out=scr_p[:, 0:length],
                        in0=em_ap,
                        in1=oh_ap,
                        op=mybir.AluOpType.mult,
                    )
                    nc.vector.tensor_reduce(
                        out=acc_ap,
                        in_=scr_p[:, 0:length],
                        axis=mybir.AxisListType.X,
                        op=mybir.AluOpType.add,
                    )

        # unshifted transitions bf16 (gold-score matmuls)
        for ic in range(JCN):
            eraw2 = stage.tile([128, T], F32, name=f"eraw2_{ic}", tag="eraw")
            nc.sync.dma_start(
                out=eraw2,
                in_=dram_ap(aux_t, AUX_TRRAW + ic * 128 * T, [[T, 128], [1, T]]),
            )
            trbf = const.tile([128, T], BF16, name=f"trbf{ic}")
            nc.vector.tensor_copy(out=trbf, in_=eraw2)
            tr_tiles.append(trbf)

        # transition part: y[j',(t)] = sum_i Tr[i,j'] OH_t[i]; dot with OH_{t+1}
        tblk = 512
        nt_calls = (length // tblk) * JCN
        acc2t = const.tile([128, nt_calls * BC], F32, name="acc2t")
        for b in range(BC):
            for tc2 in range(length // tblk):
                for jcp in range(JCN):
                    y_ps = ppool.tile([128, tblk], F32, name="y_ps", tag="pf")
                    c0 = b * LP + tc2 * tblk
                    for ic in range(JCN):
                        nc.tensor.matmul(
                            out=y_ps,
                            lhsT=tr_tiles[ic][:, jcp * 128 : (jcp + 1) * 128],
                            rhs=oh_tiles[ic][:, c0 : c0 + tblk],
                            start=(ic == 0),
                            stop=(ic == JCN - 1),
                        )
                    # drain PSUM via ACT (parallel to DVE), dot on DVE
                    ysb = stage.tile([128, tblk], BF16, name="ysb", tag="ysb")
                    nc.scalar.activation(
                        out=ysb, in_=y_ps, func=mybir.ActivationFunctionType.Copy
                    )
                    acol = (tc2 * JCN + jcp) * BC + b
                    nc.vector.scalar_tensor_tensor(
                        out=scr_v[:, 0:tblk],
                        in0=ysb,
                        scalar=1.0,
                        in1=oh_tiles[jcp][:, c0 + 1 : c0 + 1 + tblk],
                        op0=mybir.AluOpType.mult,
                        op1=mybir.AluOpType.mult,
                        accum_out=acc2t[:, acol : acol + 1],
                    )

        # fold partial sums into numacc [128, BC]
        numacc = const.tile([128, BC], F32, name="numacc")
        rede = const.tile([128, BC], F32, name="rede")
        e_view = bass.AP(
            tensor=acc2e.tensor,
            offset=acc2e.offset,
            ap=[acc2e.ap[0], [1, BC], [BC, JCN]],
        )
        nc.vector.tensor_reduce(
            out=rede, in_=e_view, axis=mybir.AxisListType.X, op=mybir.AluOpType.add
        )
        t_view = bass.AP(
            tensor=acc2t.tensor,
            offset=acc2t.offset,
            ap=[acc2t.ap[0], [1, BC], [BC, nt_calls]],
        )
        redt = const.tile([128, BC], F32, name="redt")
        nc.vector.tensor_reduce(
            out=redt, in_=t_view, axis=mybir.AxisListType.X, op=mybir.AluOpType.add
        )
        nc.vector.tensor_add(out=numacc, in0=rede, in1=redt)

        # ---------------- finalization (per chain) ----------------
        for ch in range(NCH):
            # Z (scaled) for this chain's sequences
            fin = smallp.tile([BCH, 1], F32, name=f"fin{ch}", tag="small")
            nc.tensor.matmul(
                out=fin, lhsT=prods[ch][:, 0, :], rhs=ones_col, start=True, stop=False
            )
            nc.tensor.matmul(
                out=fin, lhsT=prods[ch][:, 1, :], rhs=ones_col, start=False, stop=True
            )
            # numerator for this chain's sequences
            numer_ps = smallp.tile([BCH, 1], F32, name=f"numer_ps{ch}", tag="small")
            nc.tensor.matmul(
                out=numer_ps,
                lhsT=numacc[:, bsl[ch]],
                rhs=ones_col_f,
                start=True,
                stop=False,
            )
            for jc in range(JCN):
                oh = oh_tiles[jc]
                oh0 = bass.AP(
                    tensor=oh.tensor,
                    offset=oh.offset + ch * BCH * LP,
                    ap=[oh.ap[0], [LP, BCH]],
                )
                ohL = bass.AP(
                    tensor=oh.tensor,
                    offset=oh.offset + ch * BCH * LP + (length - 1),
                    ap=[oh.ap[0], [LP, BCH]],
                )
                nc.tensor.matmul(
                    out=numer_ps,
                    lhsT=oh0,
                    rhs=ssbf[:, jc : jc + 1],
                    start=False,
                    stop=False,
                )
                nc.tensor.matmul(
                    out=numer_ps,
                    lhsT=ohL,
                    rhs=ssbf[:, JCN + jc : JCN + jc + 1],
                    start=False,
                    stop=(jc == JCN - 1),
                )

            sumlog = const.tile([BCH, 2], F32, name=f"sumlog{ch}")
            logtmp = const.tile([BCH, nspad], F32, name=f"logtmp{ch}")
            nc.scalar.activation(
                out=logtmp,
                in_=logsf[ch],
                func=mybir.ActivationFunctionType.Ln,
                accum_out=sumlog[:, 0:1],
            )
            nc.scalar.activation(
                out=logtmp,
                in_=logsb[ch],
                func=mybir.ActivationFunctionType.Ln,
                accum_out=sumlog[:, 1:2],
            )
            logfin = const.tile([BCH, 1], F32, name=f"logfin{ch}")
            nc.scalar.activation(
                out=logfin, in_=fin, func=mybir.ActivationFunctionType.Ln
            )
            t3 = const.tile([BCH, 1], F32, name=f"t3{ch}")
            nc.vector.tensor_sub(out=t3, in0=numer_ps, in1=logfin)
            t4 = const.tile([BCH, 1], F32, name=f"t4{ch}")
            nc.vector.scalar_tensor_tensor(
                out=t4,
                in0=t3,
                scalar=float(CSH * (length - 1)),
                in1=sumlog[:, 0:1],
                op0=mybir.AluOpType.subtract,
                op1=mybir.AluOpType.subtract,
            )
            loss_sb = const.tile([BCH, 1], F32, name=f"loss_sb{ch}")
            nc.vector.tensor_sub(out=loss_sb, in0=t4, in1=sumlog[:, 1:2])
            nc.sync.dma_start(
                out=dram_ap(loss_t, ch * BCH, [[1, BCH], [1, 1]]), in_=loss_sb
            )

    nc.finalize()
    return nc


def host_inputs(inputs, tags, length=L):
    """Build per-core input maps (host-side sharding / layout prep only)."""
    inputs = np.asarray(inputs, dtype=np.float32)
    tags = np.asarray(tags)

    in_maps = []
    for c in range(NCORES):
        bsl = slice(c * BC, (c + 1) * BC)
        blk = inputs[bsl].reshape(BC, length, JCN, 128)
        # pretranspose (layout only) to [j%128, t, j//128, b] (scan) and
        # [j%128, j//128, b, t] (numerator dot-products)
        em = np.ascontiguousarray(blk.transpose(3, 1, 2, 0)).reshape(-1, 1)
        em2 = np.ascontiguousarray(blk.transpose(3, 2, 0, 1)).reshape(-1, 1)
        # tags b-major as f32 (exact for tag ids < 2^24), padded with -1
        tg = np.full((BC, LP), -1.0, dtype=np.float32)
        tg[:, :length] = tags[bsl].astype(np.float32)
        in_maps.append(dict(em=em, em2=em2, tags_bt=tg.reshape(-1, 1)))
    return in_maps


def host_shared(transitions, start_transitions, stop_transitions):
    aux = np.zeros((AUX_N, 1), dtype=np.float32)
    tr = np.asarray(transitions, dtype=np.float32)
    # shifted by -CSH: cancels between numerator gathers and log-partition
    aux[: T * T, 0] = tr.reshape(-1) - CSH
    aux[AUX_TRT : 2 * T * T, 0] = np.ascontiguousarray(tr.T).reshape(-1) - CSH
    aux[AUX_START : AUX_START + T, 0] = np.asarray(start_transitions, np.float32)
    aux[AUX_STOP : AUX_STOP + T, 0] = np.asarray(stop_transitions, np.float32)
    aux[AUX_TRRAW :, 0] = tr.reshape(-1)
    iota = np.arange(128, dtype=np.float32).reshape(128, 1)
    return dict(aux=aux, iota=iota)


def kernel(inputs, tags, mask, transitions, start_transitions, stop_transitions):
    del mask  # all-ones per the problem spec
    in_maps = host_inputs(inputs, tags)
    shared = host_shared(transitions, start_transitions, stop_transitions)
    for m_ in in_maps:
        m_.update(shared)

    nc = build_program()
    res = run_bass_kernel_spmd(nc, in_maps, core_ids=list(range(NCORES)))
    out = np.concatenate([r["loss"].reshape(BC) for r in res.results])
    return out.astype(np.float32)


if __name__ == "__main__":
    rng = np.random.default_rng(0)
    inputs = rng.standard_normal((B, L, T), dtype=np.float32)
    tags = rng.integers(0, T, size=(B, L))
    trans = rng.standard_normal((T, T)).astype(np.float32)
    start = rng.standard_normal(T).astype(np.float32)
    stop = rng.standard_normal(T).astype(np.float32)
    out = kernel(inputs, tags, np.ones((B, L), bool), trans, start, stop)
    print(out)


# revision 32
# speedup vs baseline: 1.5805x; 1.5805x over previous
"""ConditionalRandomField loss kernel for Trainium2 (8 NeuronCores).

Math (per sequence b):
    loss[b] = log_score(gold path) - log_partition
The log-partition is computed in exp space from BOTH ENDS simultaneously:
    forward   alpha_t = g_t * (E^T alpha_{t-1}),  alpha_0 = exp(start)*g_0
    backward  beta_t  = E (g_{t+1} * beta_{t+1}), beta_{L-1} = exp(stop)
    Z = alpha_m . beta_m   at the meeting point m = L/2 - 1
with E = exp(transitions - CSH), g_t = exp(emissions_t), and periodic
per-batch rescaling whose log is accumulated separately.  Halving the
sequential depth doubles the number of independent recurrence streams
(2 directions x 2 batch-chains = 4), which is what hides the per-step
PE-drain -> multiply -> matmul latency.

Engine split: forward-stream f-updates run on the Vector (DVE) engine,
backward-stream updates on the GpSimd (Pool) engine, so neither engine
serializes both directions.

The gold-path numerator uses one-hot tag masks in b-major layout
(OH[p, b*1025 + t]) so every multiply-accumulate is a contiguous
scalar_tensor_tensor; emissions are uploaded a second time in
[p, jc, b, t] order for the same reason.  All numerator work runs after
the scan (tail) so the scan's first step starts as early as possible.

NOTE: mask is all-ones for this problem spec (fill: ones); the kernel
assumes it (the reference's masked branches are identities then).
"""

import numpy as np
from contextlib import ExitStack

import concourse.bass as bass
import concourse.bacc as bacc
import concourse.tile as tile
from concourse import mybir
from concourse.bass_utils import run_bass_kernel_spmd

F32 = mybir.dt.float32
BF16 = mybir.dt.bfloat16

NCORES = 8
B = 64
L = 1024
T = 256
BC = B // NCORES      # sequences per core
PJ = 128              # partition tile of the tag dim
JCN = T // PJ         # = 2 tag chunks
RS = 32               # rescale sampling period (steps)
DEFER = 2             # rescale applied this many steps after sampling
TCH = 128             # emission-load chunk (timesteps per DMA/exp chunk)
CSH = 6.5             # constant log-shift folded into E = exp(transitions - CSH)
M = L // 2 - 1        # meeting point: fwd owns t<=M, bwd owns t>M

LP = L + 1            # per-b one-hot row width (padded col at t=L)

AUX_TRT = T * T                  # transitions^T - CSH
AUX_START = 2 * T * T
AUX_STOP = 2 * T * T + T
AUX_TRRAW = 2 * T * T + 2 * T    # unshifted transitions (numerator)
AUX_N = 3 * T * T + 2 * T

NCH = 1               # batch chains per direction (merged: width-8 matmuls)
BCH = BC // NCH


def _sample_steps(nsteps):
    # loop iterations k in [1, nsteps] at which to sample the running scale
    return [k for k in range(1, nsteps + 1) if k % RS == 0 and k + DEFER <= nsteps]


class _Bacc(bacc.Bacc):
    # Keep data waits on the MATMULs so the (data-independent) LDWEIGHTS
    # prefetch during the preceding multiply phase instead of stalling.
    def move_matmul_waits_to_ldweights(self):
        super().move_matmul_waits_to_ldweights()


def build_program(length=L, pool_bwd=True):
    """Build the single-core SPMD bass program (each core runs the same
    program on its own batch shard)."""
    assert length % 2 == 0
    m = length // 2 - 1          # fwd steps: t=1..m ; bwd steps: 1+m more
    nsamp = len(_sample_steps(m))
    nspad = max(8, ((nsamp + 7) // 8) * 8)

    nc = _Bacc()
    em_t = nc.declare_dram_parameter("em", [BC * length * T, 1], F32, isOutput=False)
    em2_t = nc.declare_dram_parameter("em2", [BC * length * T, 1], F32, isOutput=False)
    aux_t = nc.declare_dram_parameter("aux", [AUX_N, 1], F32, isOutput=False)
    tags_t = nc.declare_dram_parameter("tags_bt", [BC * LP, 1], F32, isOutput=False)
    iota_t = nc.declare_dram_parameter("iota", [128, 1], F32, isOutput=False)
    loss_t = nc.declare_dram_parameter("loss", [BC, 1], F32, isOutput=True)

    def dram_ap(handle, offset, ap):
        full = handle[:]
        return bass.AP(tensor=full.tensor, offset=offset, ap=ap)

    with tile.TileContext(nc) as tc, ExitStack() as ctx:
        const = ctx.enter_context(tc.tile_pool(name="const", bufs=1))
        stage = ctx.enter_context(tc.tile_pool(name="stage", bufs=2))
        gpool = ctx.enter_context(tc.tile_pool(name="gpool", bufs=1))
        fpool = ctx.enter_context(tc.tile_pool(name="fpool", bufs=3))
        wpool = ctx.enter_context(tc.tile_pool(name="wpool", bufs=3))
        qpool = ctx.enter_context(tc.tile_pool(name="qpool", bufs=3))
        vpool = ctx.enter_context(tc.tile_pool(name="vpool", bufs=2))
        ppool = ctx.enter_context(tc.tile_pool(name="ppool", bufs=2, space="PSUM"))
        pbpool = ctx.enter_context(tc.tile_pool(name="pbpool", bufs=2, space="PSUM"))
        spool = ctx.enter_context(tc.tile_pool(name="spool", bufs=2, space="PSUM"))
        smallp = ctx.enter_context(tc.tile_pool(name="smallp", bufs=2, space="PSUM"))

        bwd = nc.gpsimd if pool_bwd else nc.vector

        # ---------------- head emission blocks first --------------------
        # The scan's first steps only need g at the two sequence ends; load
        # tiny 16-step head blocks before anything else so the recurrences
        # start ~2us in instead of waiting for full 1MB chunks.
        gbuf = gpool.tile([128, length, JCN, BC], BF16, name="gbuf")
        row = length * JCN * BC
        HL = 16
        for hoff in (0, length - HL):
            hraw = stage.tile([128, HL, JCN, BC], F32, name="hraw", tag="raw")
            nc.sync.dma_start(
                out=hraw,
                in_=dram_ap(em_t, hoff * JCN * BC, [[row, 128], [1, HL * JCN * BC]]),
            )
            nc.scalar.activation(
                out=gbuf[:, hoff : hoff + HL, :, :],
                in_=hraw,
                func=mybir.ActivationFunctionType.Exp,
            )

        # ---------------- constants / setup ----------------
        # E = exp(transitions - CSH) / exp(transitions^T - CSH), bf16.
        e_fwd, e_bwd, tr_tiles = [], [], []
        for ic in range(JCN):
            eraw = stage.tile([128, T], F32, name=f"eraw{ic}", tag="eraw")
            nc.sync.dma_start(
                out=eraw, in_=dram_ap(aux_t, ic * 128 * T, [[T, 128], [1, T]])
            )
            ebf = const.tile([128, T], BF16, name=f"ebf{ic}")
            nc.scalar.activation(
                out=ebf, in_=eraw, func=mybir.ActivationFunctionType.Exp
            )
            e_fwd.append(ebf)
            erawt = stage.tile([128, T], F32, name=f"erawt{ic}", tag="eraw")
            nc.sync.dma_start(
                out=erawt,
                in_=dram_ap(aux_t, AUX_TRT + ic * 128 * T, [[T, 128], [1, T]]),
            )
            ebft = const.tile([128, T], BF16, name=f"ebft{ic}")
            nc.scalar.activation(
                out=ebft, in_=erawt, func=mybir.ActivationFunctionType.Exp
            )
            e_bwd.append(ebft)

        # exp(start), exp(stop) as per-partition f32 scales [128, JCN]
        ssraw = stage.tile([128, 2 * JCN], F32, name="ssraw", tag="ss")
        nc.sync.dma_start(
            out=ssraw[:, 0:JCN], in_=dram_ap(aux_t, AUX_START, [[1, 128], [128, JCN]])
        )
        nc.sync.dma_start(
            out=ssraw[:, JCN : 2 * JCN],
            in_=dram_ap(aux_t, AUX_STOP, [[1, 128], [128, JCN]]),
        )
        sstart = const.tile([128, JCN], F32, name="sstart")
        nc.scalar.activation(
            out=sstart, in_=ssraw[:, 0:JCN], func=mybir.ActivationFunctionType.Exp
        )
        sstop = const.tile([128, JCN], F32, name="sstop")
        nc.scalar.activation(
            out=sstop, in_=ssraw[:, JCN : 2 * JCN], func=mybir.ActivationFunctionType.Exp
        )
        # raw start/stop bf16 for the numerator matmuls
        ssbf = const.tile([128, 2 * JCN], BF16, name="ssbf")
        nc.vector.tensor_copy(out=ssbf, in_=ssraw)

        ones_w = const.tile([128, 128], BF16, name="ones_w")
        nc.vector.memset(ones_w, 1.0)
        ones_col = const.tile([128, 1], BF16, name="ones_col")
        nc.vector.memset(ones_col, 1.0)
        ones_col_f = const.tile([128, 1], F32, name="ones_col_f")
        nc.vector.memset(ones_col_f, 1.0)

        iota_sb = const.tile([128, 1], F32, name="iota_sb")
        nc.sync.dma_start(out=iota_sb, in_=iota_t[:])
        # tags in b-major order, padded with -1 at t=L (never matches a tag)
        tags_bt = const.tile([128, BC * LP], F32, name="tags_bt")
        nc.sync.dma_start(
            out=tags_bt, in_=dram_ap(tags_t, 0, [[0, 128], [1, BC * LP]])
        )

        # ---------------- emissions -> g = exp(emissions), bf16 ------------
        # em is host-pretransposed to [p, t, jc, b]; chunks loaded ends-first
        # so both scan directions can start early.  em2 ([p, jc, b, t]) is
        # only consumed by the tail numerator.
        em2 = gpool.tile([128, JCN, BC, length], F32, name="em2")
        nct = length // TCH
        order = []
        lo, hi = 0, nct - 1
        while lo <= hi:
            order.append(lo)
            if hi != lo:
                order.append(hi)
            lo += 1
            hi -= 1
        for tci in order:
            # skip the 16-step head blocks already loaded above
            lo_t = tci * TCH + (HL if tci == 0 else 0)
            hi_t = (tci + 1) * TCH - (HL if tci == nct - 1 else 0)
            nt = hi_t - lo_t
            raw = stage.tile([128, nt, JCN, BC], F32, name="raw", tag="raw")
            nc.sync.dma_start(
                out=raw,
                in_=dram_ap(em_t, lo_t * JCN * BC, [[row, 128], [1, nt * JCN * BC]]),
            )
            nc.scalar.activation(
                out=gbuf[:, lo_t:hi_t, :, :],
                in_=raw,
                func=mybir.ActivationFunctionType.Exp,
            )
        # em2 is contiguous per partition-row; 4 fat transfers
        for c in range(4):
            nc.sync.dma_start(
                out=bass.AP(
                    tensor=em2.tensor,
                    offset=em2.offset + c * 4 * length,
                    ap=[em2.ap[0], [1, 4 * length]],
                ),
                in_=dram_ap(em2_t, c * 4 * length, [[row, 128], [1, 4 * length]]),
            )

        # ---------------- the scan: 4 independent streams ------------------
        bsl = [slice(ch * BCH, (ch + 1) * BCH) for ch in range(NCH)]

        # forward init: alpha_0 = exp(start) * g_0
        fs = []
        for ch in range(NCH):
            f = fpool.tile([128, JCN, BCH], BF16, name=f"f{ch}", tag=f"f{ch}")
            for jc in range(JCN):
                nc.vector.tensor_scalar_mul(
                    out=f[:, jc, :],
                    in0=gbuf[:, 0, jc, bsl[ch]],
                    scalar1=sstart[:, jc : jc + 1],
                )
            fs.append(f)
        # backward init: w_0 = g_{L-1} * exp(stop)  (the TT of loop k=0)
        ws = []
        for ch in range(NCH):
            w = wpool.tile([128, JCN, BCH], BF16, name=f"w{ch}", tag=f"w{ch}")
            for jc in range(JCN):
                # DVE, not Pool: the Q7 version measured 4.1us and gated
                # the entire scan start
                nc.vector.tensor_scalar_mul(
                    out=w[:, jc, :],
                    in0=gbuf[:, length - 1, jc, bsl[ch]],
                    scalar1=sstop[:, jc : jc + 1],
                )
            ws.append(w)

        logsf = []
        logsb = []
        for ch in range(NCH):
            lf = const.tile([BCH, nspad], F32, name=f"logsf{ch}")
            nc.vector.memset(lf, 1.0)  # log(1)=0 padding
            logsf.append(lf)
            lb = const.tile([BCH, nspad], F32, name=f"logsb{ch}")
            nc.vector.memset(lb, 1.0)
            logsb.append(lb)

        def fwd_step(ch, t, f, p):
            """alpha: p = E^T f ; f' = p * g_t   (TT on vector)."""
            for jc in range(JCN):
                nc.tensor.matmul(
                    out=p[:, ch, jc, :],
                    lhsT=e_fwd[0][:, jc * 128 : (jc + 1) * 128],
                    rhs=f[:, 0, :],
                    start=True,
                    stop=False,
                )
                nc.tensor.matmul(
                    out=p[:, ch, jc, :],
                    lhsT=e_fwd[1][:, jc * 128 : (jc + 1) * 128],
                    rhs=f[:, 1, :],
                    start=False,
                    stop=True,
                )
            fn = fpool.tile([128, JCN, BCH], BF16, name=f"f{ch}", tag=f"f{ch}")
            nc.vector.tensor_tensor(
                out=fn[:],
                in0=p[:, ch],
                in1=gbuf[:, t, :, bsl[ch]],
                op=mybir.AluOpType.mult,
            )
            return fn

        def bwd_step(ch, t, w, q):
            """beta: q = E w  (w = g*beta); w' = q * g_t  (TT on pool)."""
            for ic in range(JCN):
                nc.tensor.matmul(
                    out=q[:, ch, ic, :],
                    lhsT=e_bwd[0][:, ic * 128 : (ic + 1) * 128],
                    rhs=w[:, 0, :],
                    start=True,
                    stop=False,
                )
                nc.tensor.matmul(
                    out=q[:, ch, ic, :],
                    lhsT=e_bwd[1][:, ic * 128 : (ic + 1) * 128],
                    rhs=w[:, 1, :],
                    start=False,
                    stop=True,
                )
            # With merged chains there are only 2 TTs per dual-step, so DVE
            # reads PSUM directly (no ACT hop: every cross-engine handoff
            # costs 300-600ns in standalone semaphore waits).
            wn = wpool.tile([128, JCN, BCH], BF16, name=f"w{ch}", tag=f"w{ch}")
            nc.vector.tensor_tensor(
                out=wn[:],
                in0=q[:, ch],
                in1=gbuf[:, t, :, bsl[ch]],
                op=mybir.AluOpType.mult,
            )
            return wn

        def sample(ch, x, eng, logbuf, col, gslice, dtag):
            """Sample sum(x) per sequence, store log, rescale future g."""
            s_bc = spool.tile([128, BCH], F32, name="s_bc", tag="s")
            nc.tensor.matmul(
                out=s_bc, lhsT=ones_w, rhs=x[:, 0, :], start=True, stop=False
            )
            nc.tensor.matmul(
                out=s_bc, lhsT=ones_w, rhs=x[:, 1, :], start=False, stop=True
            )
            s4 = smallp.tile([BCH, 1], F32, name="s4", tag="small")
            nc.tensor.matmul(
                out=s4, lhsT=x[:, 0, :], rhs=ones_col, start=True, stop=False
            )
            nc.tensor.matmul(
                out=s4, lhsT=x[:, 1, :], rhs=ones_col, start=False, stop=True
            )
            v = vpool.tile([128, BCH], F32, name="v", tag=f"v{dtag}{ch}")
            nc.vector.reciprocal(out=v, in_=s_bc)
            # s4 is PSUM: Pool can't read it; ACT does the copy either way
            nc.scalar.activation(
                out=logbuf[:, col : col + 1],
                in_=s4,
                func=mybir.ActivationFunctionType.Copy,
            )
            for jc in range(JCN):
                eng.tensor_mul(
                    out=gbuf[:, gslice, jc, bsl[ch]],
                    in0=gbuf[:, gslice, jc, bsl[ch]],
                    in1=v,
                )

        msamp = 0
        for k in range(1, m + 1):
            do_samp = k % RS == 0 and k + DEFER <= m
            p = ppool.tile([128, NCH, JCN, BCH], F32, name="pf", tag="pf")
            q = pbpool.tile([128, NCH, JCN, BCH], F32, name="pb", tag="pb")
            for ch in range(NCH):
                fs[ch] = fwd_step(ch, k, fs[ch], p)
                ws[ch] = bwd_step(ch, length - 1 - k, ws[ch], q)
            if do_samp:
                for ch in range(NCH):
                    sample(ch, fs[ch], nc.vector, logsf[ch], msamp, k + DEFER, "f")
                    sample(
                        ch,
                        ws[ch],
                        nc.vector,
                        logsb[ch],
                        msamp,
                        length - 1 - k - DEFER,
                        "b",
                    )
                msamp += 1
        assert msamp == nsamp

        # Final backward matmul: beta_m = E w  (no trailing multiply).
        # Total E applications: m (fwd) + m+1 (bwd) = 2m+1 = length-1.
        qfin = pbpool.tile([128, NCH, JCN, BCH], F32, name="pb", tag="pb")
        for ch in range(NCH):
            for ic in range(JCN):
                nc.tensor.matmul(
                    out=qfin[:, ch, ic, :],
                    lhsT=e_bwd[0][:, ic * 128 : (ic + 1) * 128],
                    rhs=ws[ch][:, 0, :],
                    start=True,
                    stop=False,
                )
                nc.tensor.matmul(
                    out=qfin[:, ch, ic, :],
                    lhsT=e_bwd[1][:, ic * 128 : (ic + 1) * 128],
                    rhs=ws[ch][:, 1, :],
                    start=False,
                    stop=True,
                )

        # join: Z = sum_j alpha_m[j] * beta_m[j]
        prods = []
        for ch in range(NCH):
            prod = const.tile([128, JCN, BCH], BF16, name=f"prod{ch}")
            nc.vector.tensor_tensor(
                out=prod[:], in0=qfin[:, ch], in1=fs[ch][:], op=mybir.AluOpType.mult
            )
            prods.append(prod)

        # ---------------- numerator (tail): one-hot masks -------------------
        # OH_jc[p, b*LP + t] = 1.0 iff tags[b, t] == jc*128 + p (bf16);
        # the padded tag value -1 at t=L gives a zero column automatically.
        oh_tiles = []
        for jc in range(JCN):
            oh = const.tile([128, BC * LP], BF16, name=f"oh{jc}")
            nc.vector.tensor_scalar(
                out=oh,
                in0=tags_bt,
                scalar1=float(jc * 128),
                scalar2=iota_sb[:],
                op0=mybir.AluOpType.subtract,
                op1=mybir.AluOpType.is_equal,
            )
            oh_tiles.append(oh)

        # emission part: acc2e[:, (jc,b)] = sum_t em2[p, jc, b, t]*OH[p, b*LP+t]
        ne_calls = JCN * BC
        acc2e = const.tile([128, ne_calls], F32, name="acc2e")
        scr_v = const.tile([128, length], BF16, name="scr_v")
        scr_p = const.tile([128, length], BF16, name="scr_p")
        for jc in range(JCN):
            for b in range(BC):
                em_ap = bass.AP(
                    tensor=em2.tensor,
                    offset=em2.offset + (jc * BC + b) * length,
                    ap=[em2.ap[0], [1, length]],
                )
                oh_ap = bass.AP(
                    tensor=oh_tiles[jc].tensor,
                    offset=oh_tiles[jc].offset + b * LP,
                    ap=[oh_tiles[jc].ap[0], [1, length]],
                )
                acc_ap = acc2e[:, jc * BC + b : jc * BC + b + 1]
                if b % 2 == 0:
                    nc.vector.scalar_tensor_tensor(
                        out=scr_v[:, 0:length],
                        in0=em_ap,
                        scalar=1.0,
                        in1=oh_ap,
                        op0=mybir.AluOpType.mult,
                        op1=mybir.AluOpType.mult,
                        accum_out=acc_ap,
                    )
                else:
                    # Pool has no accumulator: multiply there, reduce on DVE
                    bwd.tensor_tensor(
                        out=scr_p[:, 0:length],
                        in0=em_ap,
                        in1=oh_ap,
                        op=mybir.AluOpType.mult,
                    )
                    nc.vector.tensor_reduce(
                        out=acc_ap,
                        in_=scr_p[:, 0:length],
                        axis=mybir.AxisListType.X,
                        op=mybir.AluOpType.add,
                    )

        # unshifted transitions bf16 (gold-score matmuls)
        for ic in range(JCN):
            eraw2 = stage.tile([128, T], F32, name=f"eraw2_{ic}", tag="eraw")
            nc.sync.dma_start(
                out=eraw2,
                in_=dram_ap(aux_t, AUX_TRRAW + ic * 128 * T, [[T, 128], [1, T]]),
            )
            trbf = const.tile([128, T], BF16, name=f"trbf{ic}")
            nc.vector.tensor_copy(out=trbf, in_=eraw2)
            tr_tiles.append(trbf)

        # transition part: y[j',(t)] = sum_i Tr[i,j'] OH_t[i]; dot with OH_{t+1}
        tblk = 512
        nt_calls = (length // tblk) * JCN
        acc2t = const.tile([128, nt_calls * BC], F32, name="acc2t")
        for b in range(BC):
            for tc2 in range(length // tblk):
                for jcp in range(JCN):
                    y_ps = ppool.tile([128, tblk], F32, name="y_ps", tag="pf")
                    c0 = b * LP + tc2 * tblk
                    for ic in range(JCN):
                        nc.tensor.matmul(
                            out=y_ps,
                            lhsT=tr_tiles[ic][:, jcp * 128 : (jcp + 1) * 128],
                            rhs=oh_tiles[ic][:, c0 : c0 + tblk],
                            start=(ic == 0),
                            stop=(ic == JCN - 1),
                        )
                    # drain PSUM via ACT (parallel to DVE), dot on DVE
                    ysb = stage.tile([128, tblk], BF16, name="ysb", tag="ysb")
                    nc.scalar.activation(
                        out=ysb, in_=y_ps, func=mybir.ActivationFunctionType.Copy
                    )
                    acol = (tc2 * JCN + jcp) * BC + b
                    nc.vector.scalar_tensor_tensor(
                        out=scr_v[:, 0:tblk],
                        in0=ysb,
                        scalar=1.0,
                        in1=oh_tiles[jcp][:, c0 + 1 : c0 + 1 + tblk],
                        op0=mybir.AluOpType.mult,
                        op1=mybir.AluOpType.mult,
                        accum_out=acc2t[:, acol : acol + 1],
                    )

        # fold partial sums into numacc [128, BC]
        numacc = const.tile([128, BC], F32, name="numacc")
        rede = const.tile([128, BC], F32, name="rede")
        e_view = bass.AP(
            tensor=acc2e.tensor,
            offset=acc2e.offset,
            ap=[acc2e.ap[0], [1, BC], [BC, JCN]],
        )
        nc.vector.tensor_reduce(
            out=rede, in_=e_view, axis=mybir.AxisListType.X, op=mybir.AluOpType.add
        )
        t_view = bass.AP(
            tensor=acc2t.tensor,
            offset=acc2t.offset,
            ap=[acc2t.ap[0], [1, BC], [BC, nt_calls]],
        )
        redt = const.tile([128, BC], F32, name="redt")
        nc.vector.tensor_reduce(
            out=redt, in_=t_view, axis=mybir.AxisListType.X, op=mybir.AluOpType.add
        )
        nc.vector.tensor_add(out=numacc, in0=rede, in1=redt)

        # ---------------- finalization (per chain) ----------------
        for ch in range(NCH):
            # Z (scaled) for this chain's sequences
            fin = smallp.tile([BCH, 1], F32, name=f"fin{ch}", tag="small")
            nc.tensor.matmul(
                out=fin, lhsT=prods[ch][:, 0, :], rhs=ones_col, start=True, stop=False
            )
            nc.tensor.matmul(
                out=fin, lhsT=prods[ch][:, 1, :], rhs=ones_col, start=False, stop=True
            )
            # numerator for this chain's sequences
            numer_ps = smallp.tile([BCH, 1], F32, name=f"numer_ps{ch}", tag="small")
            nc.tensor.matmul(
                out=numer_ps,
                lhsT=numacc[:, bsl[ch]],
                rhs=ones_col_f,
                start=True,
                stop=False,
            )
            for jc in range(JCN):
                oh = oh_tiles[jc]
                oh0 = bass.AP(
                    tensor=oh.tensor,
                    offset=oh.offset + ch * BCH * LP,
                    ap=[oh.ap[0], [LP, BCH]],
                )
                ohL = bass.AP(
                    tensor=oh.tensor,
                    offset=oh.offset + ch * BCH * LP + (length - 1),
                    ap=[oh.ap[0], [LP, BCH]],
                )
                nc.tensor.matmul(
                    out=numer_ps,
                    lhsT=oh0,
                    rhs=ssbf[:, jc : jc + 1],
                    start=False,
                    stop=False,
                )
                nc.tensor.matmul(
                    out=numer_ps,
                    lhsT=ohL,
                    rhs=ssbf[:, JCN + jc : JCN + jc + 1],
                    start=False,
                    stop=(jc == JCN - 1),
                )

            sumlog = const.tile([BCH, 2], F32, name=f"sumlog{ch}")
            logtmp = const.tile([BCH, nspad], F32, name=f"logtmp{ch}")
            nc.scalar.activation(
                out=logtmp,
                in_=logsf[ch],
                func=mybir.ActivationFunctionType.Ln,
                accum_out=sumlog[:, 0:1],
            )
            nc.scalar.activation(
                out=logtmp,
                in_=logsb[ch],
                func=mybir.ActivationFunctionType.Ln,
                accum_out=sumlog[:, 1:2],
            )
            logfin = const.tile([BCH, 1], F32, name=f"logfin{ch}")
            nc.scalar.activation(
                out=logfin, in_=fin, func=mybir.ActivationFunctionType.Ln
            )
            t3 = const.tile([BCH, 1], F32, name=f"t3{ch}")
            nc.vector.tensor_sub(out=t3, in0=numer_ps, in1=logfin)
            t4 = const.tile([BCH, 1], F32, name=f"t4{ch}")
            nc.vector.scalar_tensor_tensor(
                out=t4,
                in0=t3,
                scalar=float(CSH * (length - 1)),
                in1=sumlog[:, 0:1],
                op0=mybir.AluOpType.subtract,
                op1=mybir.AluOpType.subtract,
            )
            loss_sb = const.tile([BCH, 1], F32, name=f"loss_sb{ch}")
            nc.vector.tensor_sub(out=loss_sb, in0=t4, in1=sumlog[:, 1:2])
            nc.sync.dma_start(
                out=dram_ap(loss_t, ch * BCH, [[1, BCH], [1, 1]]), in_=loss_sb
            )

    nc.finalize()
    return nc


def host_inputs(inputs, tags, length=L):
    """Build per-core input maps (host-side sharding / layout prep only)."""
    inputs = np.asarray(inputs, dtype=np.float32)
    tags = np.asarray(tags)

    in_maps = []
    for c in range(NCORES):
        bsl = slice(c * BC, (c + 1) * BC)
        blk = inputs[bsl].reshape(BC, length, JCN, 128)
        # pretranspose (layout only) to [j%128, t, j//128, b] (scan) and
        # [j%128, j//128, b, t] (numerator dot-products)
        em = np.ascontiguousarray(blk.transpose(3, 1, 2, 0)).reshape(-1, 1)
        em2 = np.ascontiguousarray(blk.transpose(3, 2, 0, 1)).reshape(-1, 1)
        # tags b-major as f32 (exact for tag ids < 2^24), padded with -1
        tg = np.full((BC, LP), -1.0, dtype=np.float32)
        tg[:, :length] = tags[bsl].astype(np.float32)
        in_maps.append(dict(em=em, em2=em2, tags_bt=tg.reshape(-1, 1)))
    return in_maps


def host_shared(transitions, start_transitions, stop_transitions):
    aux = np.zeros((AUX_N, 1), dtype=np.float32)
    tr = np.asarray(transitions, dtype=np.float32)
    # shifted by -CSH: cancels between numerator gathers and log-partition
    aux[: T * T, 0] = tr.reshape(-1) - CSH
    aux[AUX_TRT : 2 * T * T, 0] = np.ascontiguousarray(tr.T).reshape(-1) - CSH
    aux[AUX_START : AUX_START + T, 0] = np.asarray(start_transitions, np.float32)
    aux[AUX_STOP : AUX_STOP + T, 0] = np.asarray(stop_transitions, np.float32)
    aux[AUX_TRRAW :, 0] = tr.reshape(-1)
    iota = np.arange(128, dtype=np.float32).reshape(128, 1)
    return dict(aux=aux, iota=iota)


def kernel(inputs, tags, mask, transitions, start_transitions, stop_transitions):
    del mask  # all-ones per the problem spec
    in_maps = host_inputs(inputs, tags)
    shared = host_shared(transitions, start_transitions, stop_transitions)
    for m_ in in_maps:
        m_.update(shared)

    nc = build_program()
    res = run_bass_kernel_spmd(nc, in_maps, core_ids=list(range(NCORES)))
    out = np.concatenate([r["loss"].reshape(BC) for r in res.results])
    return out.astype(np.float32)


if __name__ == "__main__":
    rng = np.random.default_rng(0)
    inputs = rng.standard_normal((B, L, T), dtype=np.float32)
    tags = rng.integers(0, T, size=(B, L))
    trans = rng.standard_normal((T, T)).astype(np.float32)
    start = rng.standard_normal(T).astype(np.float32)
    stop = rng.standard_normal(T).astype(np.float32)
    out = kernel(inputs, tags, np.ones((B, L), bool), trans, start, stop)
    print(out)
